# revision 23
# baseline (speedup 1.0000x reference)
"""Trainium2 Bass kernel for the ENAS-style controller sampler.

Single-core fused kernel (replicated SPMD on 8 cores; the problem is fully
sequential and not shardable).  All weights live in SBUF.

Per sampling step:
  - LSTM gates = two accumulating matmul groups into one PSUM (100,4):
      statB: 4x (W_hh_G.T ; bias) x [2h; 1]   -- issued EARLY so they
          overlap the previous step's sampling tail
      EW: ONE block-diagonal matmul (K=128, N=4): stationary rows
          32j:32j+14 hold gate j's emb @ W_ih.T slice; moving is a
          block-diagonal one-hot (128,4) -- finishes all 4 gates in a
          single instruction once the sample is known
  - sigmoid-free LSTM algebra on doubled states (C=2c, H=2h):
      2*sig(x) = 1 + tanh(x/2)  (i,f,o weights pre-halved host-side)
      C' = 0.5*(1+th_f)*C + (1+th_i)*th_g ;  H = (1+th_o)*tanh(C'/2)
      consumers of h (W_hh, W_hid_attn, W_soft) pre-halved host-side
  - attention logits fused: [0.5*W_hid_attn.T ; E_att] @ [H ; e_j] via a
      replicated-H + identity moving tile
  - softmax in log-space (logits bounded +-2.5, no max-subtract);
      exp via a Chebyshev/Horner chain on the vector engine (ping-pong
      buffers); inverse-CDF sample via cumsum scan + is_lt w/ fused count
  - one-hot rebuild: tiny ones-matmul broadcast + 4 iota-compares writing
      the block-diagonal one-hot columns
  - all 20 ln(sum_exp) deferred to one end-of-kernel Ln over a (1,20) row
"""

import os
import sys
import numpy as np

if "/opt/trn_rl_repo" not in sys.path:
    sys.path.insert(0, "/opt/trn_rl_repo")

N_NODES = 5
N_OPS = 8
HID = 100
VOCAB = N_NODES + 1 + N_OPS  # 14
TEMP = 5.0
TANH_C = 2.5
OP_TANH_REDUCE = 2.5

SHIFT_OH = True  # DVE writes one-hot at partition bases 32j from base-0 pb


def _exp_chain_coeffs(a, deg):
    """Even/odd (Estrin-2) coefficient split for exp(a*t) - 1 on t in [-1,1].

    exp(a*t) - 1 = sum_m q_m t^m  (m = 1..deg+1, Chebyshev fit of
    (e^x-1)/x on [-a,a]).  Returns (ev, od, q1) where the two Horner
    chains run in t2 = t*t:
        EV  = sum_n q_{2n}   t2^n   (chain coeffs ev, high->low)
        ODm = sum_n q_{2n+1} t2^n   (n>=1, chain coeffs od, high->low)
        e - 1 = EV + ((ODm + q1) * t)
    """
    x = np.cos(np.pi * (np.arange(4000) + 0.5) / 4000) * a
    ch = np.polynomial.chebyshev.Chebyshev.fit(
        x, np.expm1(x) / x, deg, domain=[-a, a])
    c = ch.convert(kind=np.polynomial.Polynomial).coef  # ascending in x
    q = {m: float(c[m - 1] * a ** m) for m in range(1, len(c) + 1)}
    top = max(q)
    ev = [q[m] for m in range(top if top % 2 == 0 else top - 1, 1, -2)]
    od = [q[m] for m in range(top if top % 2 == 1 else top - 1, 2, -2)]
    return ev, od, q[1]


_EXPC_NODE = _exp_chain_coeffs(TANH_C, 10)
_EXPC_OP = _exp_chain_coeffs(TANH_C / OP_TANH_REDUCE, 6)

# ---- constant blob layout (128 x CW fp32) ----
# cols 0:400     statB tiles, psum col order [i, f, o, g]; each (101,100)
#                rows 0:100 = 0.5*hsc*W_hh_G.T, row 100 = hsc*(b_ih+b_hh)_G
# cols 400:500   [0.5*W_hid_attn.T ; E_att[0:6]]   (106,100)
# cols 500:508   [0.5*W_soft.T ; b_soft]       (101,8)
# col  514       v_attn                    (100,1)
# col  515       iota_node: rows 32j+i = i
# col  516       iota_op:   rows 32j+i = i-6
# cols 517:537   1/u                       (1,20)
# cols 551:559   zeros (scan data1 row)
# cols 568:678   ones row for action broadcast
# cols 678:684   h2rep init: rows 0:100 zero, rows 100:106 = I6
CW = 684
C_STAT = 0
C_WHID = 400
C_WSOFT = 500
C_V = 514
C_IOTA_N = 515
C_IOTA_O = 516
C_U = 517
C_ZERO = 551
C_ONES = 568
C_CSV = 600
C_H2R = 678

# cstA = (128, 100) block stationary: rows 32j+i = hsc_j * EW[i, gate_j]


def _host_constants(emb, W_emb_attn, W_hid_attn, v_attn, W_soft, b_soft,
                    W_ih, W_hh, b_ih, b_hh, u):
    cst = np.zeros((128, CW), np.float32)
    cstA = np.zeros((128, 100), np.float32)
    EW = (emb @ W_ih.T).astype(np.float32)          # (14,400)
    bsum = (b_ih + b_hh).astype(np.float32)         # (400,)
    # reference gate slices: i=[0:100] f=[100:200] g=[200:300] o=[300:400]
    # psum column order [i, f, o, g]
    for j, gi in enumerate([0, 1, 3, 2]):
        sl = slice(gi * 100, (gi + 1) * 100)
        # halve i,f,o pre-activations: 2*sigmoid(x) = 1 + tanh(x/2)
        hsc = 0.5 if j < 3 else 1.0
        cstA[32 * j:32 * j + 14, 0:100] = hsc * EW[:, sl]
        # extra 0.5 on recurrent weights: h is stored doubled (H = 2h)
        cst[0:100, j * 100:(j + 1) * 100] = 0.5 * hsc * W_hh[sl, :].T
        cst[100, j * 100:(j + 1) * 100] = hsc * bsum[sl]
    cst[0:100, C_WHID:C_WHID + 100] = 0.5 * W_hid_attn.T
    E_att = (emb @ W_emb_attn.T).astype(np.float32)  # (14,100)
    cst[100:106, C_WHID:C_WHID + 100] = E_att[0:6, :]
    cst[0:100, C_WSOFT:C_WSOFT + 8] = 0.5 * W_soft.T
    cst[100, C_WSOFT:C_WSOFT + 8] = b_soft
    cst[0:100, C_V] = v_attn
    for j in range(4):
        cst[32 * j:32 * j + 14, C_IOTA_N] = np.arange(14, dtype=np.float32)
        cst[32 * j:32 * j + 14, C_IOTA_O] = (np.arange(14, dtype=np.float32)
                                             - (N_NODES + 1))
    cst[0, C_U:C_U + 20] = (1.0 / u).astype(np.float32)
    cst[0, C_ONES:C_ONES + 110] = 1.0
    csv = []
    for _ in range(N_NODES):
        csv += [TANH_C, TANH_C, TANH_C / OP_TANH_REDUCE,
                TANH_C / OP_TANH_REDUCE]
    cst[0, C_CSV:C_CSV + 20] = csv
    cst[100:106, C_H2R:C_H2R + 6] = np.eye(6, dtype=np.float32)
    return {"cst": cst, "cstA": cstA}


def _build():
    import concourse.bass as bass
    from concourse.bass import _add_dep_helper
    import concourse.bacc as bacc
    import concourse.tile as tile
    from concourse import mybir

    f32 = mybir.dt.float32
    AF = mybir.ActivationFunctionType
    OP = mybir.AluOpType

    nc = bacc.Bacc("TRN2", target_bir_lowering=False, debug=False,
                   num_devices=8)
    cst_d = nc.declare_dram_parameter("cst", [128, CW], f32, isOutput=False)
    cstA_d = nc.declare_dram_parameter("cstA", [128, 100], f32,
                                       isOutput=False)
    out_d = nc.declare_dram_parameter("out", [1, 24], f32, isOutput=True)

    with tile.TileContext(nc) as tc:
        with (
            tc.tile_pool(name="sb", bufs=1) as sb,
            tc.tile_pool(name="ps", bufs=1, space="PSUM") as ps,
        ):
            cst = sb.tile([128, CW], f32)
            cstA = sb.tile([128, 100], f32)
            nc.sync.dma_start(out=cst[:, :], in_=cst_d[:, :])
            nc.sync.dma_start(out=cstA[:, :], in_=cstA_d[:, :])

            ohB = sb.tile([128, 4], f32)    # block-diagonal one-hot
            h2rep = sb.tile([106, 6], f32)  # [2h x6 ; I6]
            c = sb.tile([100, 1], f32)      # C = 2c
            th4 = sb.tile([100, 4], f32)    # tanh of (i/2, f/2, o/2, g)
            tg = sb.tile([100, 1], f32)     # P2 then tanh(c2)
            m1 = sb.tile([100, 1], f32)     # P1
            s = sb.tile([100, 6], f32)
            t_row = sb.tile([1, 8], f32)
            t2 = sb.tile([1, 8], f32)       # t*t for the Estrin chains
            pa = sb.tile([1, 8], f32)       # even-power chain
            pc = sb.tile([1, 8], f32)       # odd-power chain
            px = sb.tile([1, 8], f32)       # (ODm + q1) * t
            e_row = sb.tile([1, 8], f32)
            cumX = sb.tile([1, 9], f32)     # col 0 = 0, cols 1..8 = cumsum(e)
            oh = sb.tile([1, 8], f32)
            Tn = sb.tile([1, 15], f32)
            To = sb.tile([1, 9], f32)
            sumes = sb.tile([1, 20], f32)   # per-step sum(exp)
            ta20 = sb.tile([1, 20], f32)    # per-step t[action]
            tsd20 = sb.tile([1, 20], f32)   # per-step sum(e*t)
            lns = sb.tile([1, 20], f32)
            rec20 = sb.tile([1, 20], f32)
            q20 = sb.tile([1, 20], f32)
            lnt = sb.tile([1, 1], f32)
            atp = sb.tile([1, 1], f32)
            enp = sb.tile([1, 1], f32)
            jk1 = sb.tile([1, 8], f32)
            jk2 = sb.tile([1, 8], f32)
            jk3 = sb.tile([1, 20], f32)
            outb = sb.tile([1, 24], f32)

            pg = ps.tile([100, 4], f32)
            psp = ps.tile([100, 6], f32)
            pl = ps.tile([1, 8], f32)
            pb = ps.tile([110, 1], f32)

            V, S, TE = nc.vector, nc.scalar, nc.tensor

            V.memset(ohB[:, :], 0.0)
            nc.sync.dma_start(out=h2rep[0:106, 0:6],
                              in_=cst_d[0:106, C_H2R:C_H2R + 6])
            V.memset(c[:, :], 0.0)
            V.memset(Tn[:, :], 0.0)
            V.memset(Tn[0:1, 0:1], 1.0)
            V.memset(To[:, :], 0.0)
            V.memset(To[0:1, 0:1], 1.0)
            V.memset(outb[:, :], 0.0)
            V.memset(cumX[0:1, 0:1], 0.0)

            def emit_statB(after=None):
                # start=True only on the first: it arms pending-zero for the
                # whole PSUM region, so the other columns lazily zero on
                # their first write (a second start would re-arm the region
                # and discard already-written columns)
                for j in range(4):
                    r = TE.matmul(pg[0:100, j:j + 1],
                                  cst[0:101, C_STAT + j * 100:C_STAT + (j + 1) * 100],
                                  h2rep[0:101, 0:1], start=(j == 0),
                                  stop=False, skip_group_check=True)
                    if j == 0 and after is not None:
                        # keep the 1.5us recurrent block behind this step's
                        # logits matmul on the PE queue (greedy scheduler
                        # otherwise wedges it in front)
                        _add_dep_helper(r.ins, after.ins, sync=False,
                                        reason="statB after logits")

            emit_statB()  # step 0: h = 0 (h2rep DMA init), one-hot = 0

            prev_ctx = [None]

            G = nc.gpsimd

            def emit_offpath():
                px_ = prev_ctx[0]
                if px_ is None:
                    return
                k, step, Cs, T, last_iseq = px_
                # true e values (cum diff; the chains left e-1 in e_row)
                G.tensor_tensor(e_row[0:1, 0:k], cumX[0:1, 1:1 + k],
                                cumX[0:1, 0:k], op=OP.subtract)
                G.tensor_tensor(oh[0:1, 0:k], T[0:1, 0:k], T[0:1, 1:k + 1],
                                op=OP.subtract)
                G.tensor_copy(sumes[0:1, step:step + 1], cumX[0:1, k:k + 1])
                V.scalar_tensor_tensor(jk1[0:1, 0:k], oh[0:1, 0:k], 1.0,
                                       t_row[0:1, 0:k], op0=OP.mult,
                                       op1=OP.mult,
                                       accum_out=ta20[0:1, step:step + 1])
                V.scalar_tensor_tensor(jk2[0:1, 0:k], e_row[0:1, 0:k], 1.0,
                                       t_row[0:1, 0:k], op0=OP.mult,
                                       op1=OP.mult,
                                       accum_out=tsd20[0:1, step:step + 1])
                prev_ctx[0] = None

            def emit_step(kind, k, pos, step):
                Cs = TANH_C if kind == "node" else TANH_C / OP_TANH_REDUCE
                T = Tn if kind == "node" else To
                iota_col = C_IOTA_N if kind == "node" else C_IOTA_O

                # ---- finish LSTM gates: one block-diagonal one-hot MM ----
                TE.matmul(pg[0:100, 0:4], cstA[0:128, 0:100],
                          ohB[0:128, 0:4], start=False, stop=True,
                          skip_group_check=True)

                # previous step's log_p/entropy bookkeeping runs on DVE
                # while PE/ACT work above
                emit_offpath()

                # ---- sigmoid-free LSTM (C = 2c, H = 2h) ----
                S.activation(th4[:, :], pg[0:100, 0:4], AF.Tanh)
                V.scalar_tensor_tensor(m1[:, :], th4[0:100, 0:1], 1.0,
                                       th4[0:100, 3:4], op0=OP.add,
                                       op1=OP.mult)   # P1 = (1+th_i)*th_g
                V.scalar_tensor_tensor(tg[:, :], th4[0:100, 1:2], 1.0,
                                       c[:, :], op0=OP.add,
                                       op1=OP.mult)   # P2 = (1+th_f)*C
                V.scalar_tensor_tensor(c[:, :], tg[:, :], 0.5, m1[:, :],
                                       op0=OP.mult, op1=OP.add)  # C'
                S.activation(tg[:, :], c[:, :], AF.Tanh, scale=0.5)
                V.scalar_tensor_tensor(
                    h2rep[0:100, 0:6],
                    th4[0:100, 2:3].broadcast_to([100, 6]), 1.0,
                    tg[0:100, 0:1].broadcast_to([100, 6]),
                    op0=OP.add, op1=OP.mult)  # H = (1+th_o)*tanh(C'/2)

                # ---- logits ----
                if kind == "node":
                    TE.matmul(psp[0:100, 0:k],
                              cst[0:100 + k, C_WHID:C_WHID + 100],
                              h2rep[0:100 + k, 0:k],
                              start=True, stop=True, skip_group_check=True)
                    S.activation(s[0:100, 0:k], psp[0:100, 0:k], AF.Tanh)
                    rl = TE.matmul(pl[0:1, 0:k], cst[0:100, C_V:C_V + 1],
                                   s[0:100, 0:k], start=True, stop=True,
                                   skip_group_check=True)
                else:
                    rl = TE.matmul(pl[0:1, 0:8], h2rep[0:101, 0:1],
                                   cst[0:101, C_WSOFT:C_WSOFT + 8],
                                   start=True, stop=True,
                                   skip_group_check=True)

                # ---- prefetch next step's recurrent gate matmuls ----
                if step < 19:
                    emit_statB(after=rl)

                # ---- softmax (log-space) + inverse-CDF sample ----
                S.activation(t_row[0:1, 0:k], pl[0:1, 0:k], AF.Tanh,
                             scale=1.0 / TEMP)
                # exp(Cs*t) - 1 via two interleaved Horner chains in t^2
                ev, od, q1 = _EXPC_NODE if kind == "node" else _EXPC_OP
                V.tensor_tensor(t2[0:1, 0:k], t_row[0:1, 0:k],
                                t_row[0:1, 0:k], op=OP.mult)
                V.tensor_scalar(pa[0:1, 0:k], t2[0:1, 0:k], ev[0], None,
                                op0=OP.mult)
                V.tensor_scalar(pc[0:1, 0:k], t2[0:1, 0:k], od[0], None,
                                op0=OP.mult)
                for i in range(1, max(len(ev), len(od))):
                    if i < len(ev):
                        V.scalar_tensor_tensor(pa[0:1, 0:k], pa[0:1, 0:k],
                                               ev[i], t2[0:1, 0:k],
                                               op0=OP.add, op1=OP.mult)
                    if i < len(od):
                        V.scalar_tensor_tensor(pc[0:1, 0:k], pc[0:1, 0:k],
                                               od[i], t2[0:1, 0:k],
                                               op0=OP.add, op1=OP.mult)
                V.scalar_tensor_tensor(px[0:1, 0:k], pc[0:1, 0:k], q1,
                                       t_row[0:1, 0:k], op0=OP.add,
                                       op1=OP.mult)
                V.tensor_tensor(e_row[0:1, 0:k], pa[0:1, 0:k], px[0:1, 0:k],
                                op=OP.add)  # e - 1
                # cumsum of (e-1) + 1 in one scan; col 0 stays 0
                V.tensor_tensor_scan(cumX[0:1, 1:1 + k], e_row[0:1, 0:k],
                                     cst[0:1, C_ONES:C_ONES + k], 0.0,
                                     op0=OP.add, op1=OP.add)
                # indicator cum_j * (1/u) < sum_e, with fused count
                V.scalar_tensor_tensor(
                    T[0:1, 1:k], cumX[0:1, 1:k],
                    cst[0:1, C_U + step:C_U + step + 1],
                    cumX[0:1, k:k + 1].broadcast_to([1, k - 1]),
                    op0=OP.mult, op1=OP.is_lt,
                    accum_out=outb[0:1, pos:pos + 1])

                # ---- fold action back in as block-diagonal one-hot ----
                TE.matmul(pb[0:14, 0:1], cst[0:1, C_ONES:C_ONES + 14],
                          outb[0:1, pos:pos + 1], start=True, stop=True,
                          skip_group_check=True)
                last = None
                for j in range(4):
                    last = V.tensor_tensor(ohB[32 * j:32 * j + 14, j:j + 1],
                                           cst[0:14, iota_col:iota_col + 1],
                                           pb[0:14, 0:1], op=OP.is_equal)

                prev_ctx[0] = (k, step, Cs, T, last)

            step = 0
            for node_idx in range(N_NODES):
                for i in range(2):
                    emit_step("node", node_idx + 2, node_idx * 2 + i, step)
                    step += 1
                for i in range(2):
                    emit_step("op", 8, 10 + node_idx * 2 + i, step)
                    step += 1
            emit_offpath()

            # log_p  = sum_s Cs*t_a[s] - sum_s ln(sume[s])
            # entropy = sum_s ln(sume[s]) - sum_s Cs*tsd[s]/sume[s]
            S.activation(lns[0:1, 0:20], sumes[0:1, 0:20], AF.Ln,
                         accum_out=lnt[0:1, 0:1])
            V.scalar_tensor_tensor(jk3[0:1, 0:20], ta20[0:1, 0:20], 1.0,
                                   cst[0:1, C_CSV:C_CSV + 20], op0=OP.mult,
                                   op1=OP.mult, accum_out=atp[0:1, 0:1])
            V.reciprocal(rec20[0:1, 0:20], sumes[0:1, 0:20])
            V.tensor_tensor(q20[0:1, 0:20], tsd20[0:1, 0:20],
                            rec20[0:1, 0:20], op=OP.mult)
            V.scalar_tensor_tensor(jk3[0:1, 0:20], q20[0:1, 0:20], 1.0,
                                   cst[0:1, C_CSV:C_CSV + 20], op0=OP.mult,
                                   op1=OP.mult, accum_out=enp[0:1, 0:1])
            V.tensor_tensor(outb[0:1, 20:21], atp[:, :], lnt[:, :],
                            op=OP.subtract)
            V.tensor_tensor(outb[0:1, 21:22], lnt[:, :], enp[:, :],
                            op=OP.subtract)

            nc.sync.dma_start(out=out_d[:, :], in_=outb[0:1, 0:24])

    nc.compile()
    return nc


_NC_CACHE = {}


def _get_nc():
    if "nc" not in _NC_CACHE:
        _NC_CACHE["nc"] = _build()
    return _NC_CACHE["nc"]


def _run(inputs, trace=False):
    from concourse.bass_utils import run_bass_kernel_spmd

    blobs = _host_constants(**{k: np.asarray(v, np.float32)
                               for k, v in inputs.items()})
    nc = _get_nc()
    core_ids = list(range(8))
    in_maps = [dict(blobs) for _ in core_ids]
    res = run_bass_kernel_spmd(nc, in_maps, core_ids, trace=trace)
    return res


def _unpack(out):
    out = np.asarray(out).reshape(-1)
    prev_nodes = np.rint(out[0:10]).astype(np.int32)
    prev_ops = np.rint(out[10:20]).astype(np.int32)
    log_p = np.float32(out[20])
    entropy = np.float32(out[21])
    return prev_nodes, prev_ops, log_p, entropy


def kernel(**inputs):
    res = _run(inputs, trace=False)
    return _unpack(res.results[0]["out"])


# revision 29
# speedup vs baseline: 1.2238x; 1.2238x over previous
"""Trainium2 Bass kernel for the ENAS-style controller sampler.

Single-core fused kernel (replicated SPMD on 8 cores; the problem is fully
sequential and not shardable).  All weights live in SBUF.

Per sampling step:
  - LSTM gates = two accumulating matmul groups into one PSUM (100,4):
      statB: 4x (W_hh_G.T ; bias) x [2h; 1]   -- issued EARLY so they
          overlap the previous step's sampling tail
      EW: ONE block-diagonal matmul (K=128, N=4): stationary rows
          32j:32j+14 hold gate j's emb @ W_ih.T slice; moving is a
          block-diagonal one-hot (128,4) -- finishes all 4 gates in a
          single instruction once the sample is known
  - sigmoid-free LSTM algebra on doubled states (C=2c, H=2h):
      2*sig(x) = 1 + tanh(x/2)  (i,f,o weights pre-halved host-side)
      C' = 0.5*(1+th_f)*C + (1+th_i)*th_g ;  H = (1+th_o)*tanh(C'/2)
      consumers of h (W_hh, W_hid_attn, W_soft) pre-halved host-side
  - attention logits fused: [0.5*W_hid_attn.T ; E_att] @ [H ; e_j] via a
      replicated-H + identity moving tile
  - softmax in log-space (logits bounded +-2.5, no max-subtract);
      exp via a Chebyshev/Horner chain on the vector engine (ping-pong
      buffers); inverse-CDF sample via cumsum scan + is_lt w/ fused count
  - one-hot rebuild: tiny ones-matmul broadcast + 4 iota-compares writing
      the block-diagonal one-hot columns
  - all 20 ln(sum_exp) deferred to one end-of-kernel Ln over a (1,20) row
"""

import os
import sys
import numpy as np

if "/opt/trn_rl_repo" not in sys.path:
    sys.path.insert(0, "/opt/trn_rl_repo")

N_NODES = 5
N_OPS = 8
HID = 100
VOCAB = N_NODES + 1 + N_OPS  # 14
TEMP = 5.0
TANH_C = 2.5
OP_TANH_REDUCE = 2.5

SHIFT_OH = True  # DVE writes one-hot at partition bases 32j from base-0 pb
DUMMY_DEP = True  # force statB behind node logits via a real cst[0,0] write
JK_DEP = True     # push the jk bookkeeping dots behind the is_eq block
SLIM_TAIL = True  # drop the second all-engine barrier in the Tile epilogue


def _exp_chain_coeffs(a, deg):
    """Even/odd (Estrin-2) coefficient split for exp(a*t) - 1 on t in [-1,1].

    exp(a*t) - 1 = sum_m q_m t^m  (m = 1..deg+1, Chebyshev fit of
    (e^x-1)/x on [-a,a]).  Returns (ev, od, q1) where the two Horner
    chains run in t2 = t*t:
        EV  = sum_n q_{2n}   t2^n   (chain coeffs ev, high->low)
        ODm = sum_n q_{2n+1} t2^n   (n>=1, chain coeffs od, high->low)
        e - 1 = EV + ((ODm + q1) * t)
    """
    x = np.cos(np.pi * (np.arange(4000) + 0.5) / 4000) * a
    ch = np.polynomial.chebyshev.Chebyshev.fit(
        x, np.expm1(x) / x, deg, domain=[-a, a])
    c = ch.convert(kind=np.polynomial.Polynomial).coef  # ascending in x
    q = {m: float(c[m - 1] * a ** m) for m in range(1, len(c) + 1)}
    top = max(q)
    ev = [q[m] for m in range(top if top % 2 == 0 else top - 1, 1, -2)]
    od = [q[m] for m in range(top if top % 2 == 1 else top - 1, 2, -2)]
    return ev, od, q[1]


_EXPC_NODE = _exp_chain_coeffs(TANH_C, 10)
_EXPC_OP = _exp_chain_coeffs(TANH_C / OP_TANH_REDUCE, 6)

# ---- constant blob layout (128 x CW fp32) ----
# cols 0:400     statB tiles, psum col order [i, f, o, g]; each (101,100)
#                rows 0:100 = 0.5*hsc*W_hh_G.T, row 100 = hsc*(b_ih+b_hh)_G
# cols 400:500   [0.5*W_hid_attn.T ; E_att[0:6]]   (106,100)
# cols 500:508   [0.5*W_soft.T ; b_soft]       (101,8)
# col  514       v_attn                    (100,1)
# col  515       iota_node: rows 32j+i = i
# col  516       iota_op:   rows 32j+i = i-6
# cols 517:537   1/u                       (1,20)
# cols 551:559   zeros (scan data1 row)
# cols 568:678   ones row for action broadcast
# cols 678:684   h2rep init: rows 0:100 zero, rows 100:106 = I6
CW = 684
C_STAT = 0
C_WHID = 400
C_WSOFT = 500
C_V = 514
C_IOTA_N = 515
C_IOTA_O = 516
C_U = 517
C_ZERO = 551
C_ONES = 568
C_CSV = 600
C_H2R = 678

# cstA = (128, 100) block stationary: rows 32j+i = hsc_j * EW[i, gate_j]


def _host_constants(emb, W_emb_attn, W_hid_attn, v_attn, W_soft, b_soft,
                    W_ih, W_hh, b_ih, b_hh, u):
    cst = np.zeros((128, CW), np.float32)
    cstA = np.zeros((128, 100), np.float32)
    EW = (emb @ W_ih.T).astype(np.float32)          # (14,400)
    bsum = (b_ih + b_hh).astype(np.float32)         # (400,)
    # reference gate slices: i=[0:100] f=[100:200] g=[200:300] o=[300:400]
    # psum column order [i, f, o, g]
    for j, gi in enumerate([0, 1, 3, 2]):
        sl = slice(gi * 100, (gi + 1) * 100)
        # halve i,f,o pre-activations: 2*sigmoid(x) = 1 + tanh(x/2)
        hsc = 0.5 if j < 3 else 1.0
        cstA[32 * j:32 * j + 14, 0:100] = hsc * EW[:, sl]
        # extra 0.5 on recurrent weights: h is stored doubled (H = 2h)
        cst[0:100, j * 100:(j + 1) * 100] = 0.5 * hsc * W_hh[sl, :].T
        cst[100, j * 100:(j + 1) * 100] = hsc * bsum[sl]
    cst[0:100, C_WHID:C_WHID + 100] = 0.5 * W_hid_attn.T
    E_att = (emb @ W_emb_attn.T).astype(np.float32)  # (14,100)
    cst[100:106, C_WHID:C_WHID + 100] = E_att[0:6, :]
    cst[0:100, C_WSOFT:C_WSOFT + 8] = 0.5 * W_soft.T
    cst[100, C_WSOFT:C_WSOFT + 8] = b_soft
    cst[0:100, C_V] = v_attn
    for j in range(4):
        cst[32 * j:32 * j + 14, C_IOTA_N] = np.arange(14, dtype=np.float32)
        cst[32 * j:32 * j + 14, C_IOTA_O] = (np.arange(14, dtype=np.float32)
                                             - (N_NODES + 1))
    cst[0, C_U:C_U + 20] = (1.0 / u).astype(np.float32)
    cst[0, C_ONES:C_ONES + 110] = 1.0
    csv = []
    for _ in range(N_NODES):
        csv += [TANH_C, TANH_C, TANH_C / OP_TANH_REDUCE,
                TANH_C / OP_TANH_REDUCE]
    cst[0, C_CSV:C_CSV + 20] = csv
    cst[100:106, C_H2R:C_H2R + 6] = np.eye(6, dtype=np.float32)
    return {"cst": cst, "cstA": cstA}


def _build():
    import concourse.bass as bass
    from concourse.bass import _add_dep_helper
    import concourse.bacc as bacc
    import concourse.tile as tile
    from concourse import mybir

    f32 = mybir.dt.float32
    AF = mybir.ActivationFunctionType
    OP = mybir.AluOpType

    if SLIM_TAIL and not getattr(tile.TileContext, "_slim_tail", False):
        _orig_dab = tile.TileContext._drain_and_barrier

        def _slim_dab(self, tick_clock, wait_clock):
            from concourse.tile import ScopedClock
            drain_inst = self.nc.sync.drain()
            wait_clock.add_sem_waits(
                drain_inst.ins, ScopedClock({None: tick_clock.global_clock}))
            self.nc.all_engine_barrier()
            popped = self.nc._tile_sem_poison_stack.pop()
            assert popped is self._sem_poison
            self.nc.clear_and_free_semaphores(
                list(self.sems.allocated().values()))
            # second all_engine_barrier skipped: the NEFF end already waits
            # for every queue, and the sem clears are engine instructions

        tile.TileContext._drain_and_barrier = _slim_dab
        tile.TileContext._slim_tail = True

    nc = bacc.Bacc("TRN2", target_bir_lowering=False, debug=False,
                   num_devices=8)
    cst_d = nc.declare_dram_parameter("cst", [128, CW], f32, isOutput=False)
    cstA_d = nc.declare_dram_parameter("cstA", [128, 100], f32,
                                       isOutput=False)
    out_d = nc.declare_dram_parameter("out", [1, 24], f32, isOutput=True)

    with tile.TileContext(nc) as tc:
        with (
            tc.tile_pool(name="sb", bufs=1) as sb,
            tc.tile_pool(name="ps", bufs=1, space="PSUM") as ps,
        ):
            cst = sb.tile([128, CW], f32)
            cstA = sb.tile([128, 100], f32)
            nc.sync.dma_start(out=cst[:, :], in_=cst_d[:, :])
            nc.sync.dma_start(out=cstA[:, :], in_=cstA_d[:, :])

            ohB = sb.tile([128, 4], f32)    # block-diagonal one-hot
            h2rep = sb.tile([106, 6], f32)  # [2h x6 ; I6]
            c = sb.tile([100, 1], f32)      # C = 2c
            th4 = sb.tile([100, 4], f32)    # tanh of (i/2, f/2, o/2, g)
            tg = sb.tile([100, 1], f32)     # P2 then tanh(c2)
            m1 = sb.tile([100, 1], f32)     # P1
            s = sb.tile([100, 6], f32)
            t_row = sb.tile([1, 8], f32)
            t2 = sb.tile([1, 8], f32)       # t*t for the Estrin chains
            pa = sb.tile([1, 8], f32)       # even-power chain
            pc = sb.tile([1, 8], f32)       # odd-power chain
            px = sb.tile([1, 8], f32)       # (ODm + q1) * t
            e_row = sb.tile([1, 8], f32)
            cumX = sb.tile([1, 9], f32)     # col 0 = 0, cols 1..8 = cumsum(e)
            oh = sb.tile([1, 8], f32)
            Tn = sb.tile([1, 15], f32)
            To = sb.tile([1, 9], f32)
            sumes = sb.tile([1, 20], f32)   # per-step sum(exp)
            ta20 = sb.tile([1, 20], f32)    # per-step t[action]
            tsd20 = sb.tile([1, 20], f32)   # per-step sum(e*t)
            lns = sb.tile([1, 20], f32)
            rec20 = sb.tile([1, 20], f32)
            q20 = sb.tile([1, 20], f32)
            lnt = sb.tile([1, 1], f32)
            atp = sb.tile([1, 1], f32)
            enp = sb.tile([1, 1], f32)
            jk1 = sb.tile([1, 8], f32)
            jk2 = sb.tile([1, 8], f32)
            jk3 = sb.tile([1, 20], f32)
            outb = sb.tile([1, 24], f32)

            pg = ps.tile([100, 4], f32)
            psp = ps.tile([100, 6], f32)
            pl = ps.tile([1, 8], f32)
            pb = ps.tile([110, 1], f32)

            V, S, TE = nc.vector, nc.scalar, nc.tensor

            V.memset(ohB[:, :], 0.0)
            nc.sync.dma_start(out=h2rep[0:106, 0:6],
                              in_=cst_d[0:106, C_H2R:C_H2R + 6])
            V.memset(c[:, :], 0.0)
            V.memset(Tn[:, :], 0.0)
            V.memset(Tn[0:1, 0:1], 1.0)
            V.memset(To[:, :], 0.0)
            V.memset(To[0:1, 0:1], 1.0)
            V.memset(outb[:, :], 0.0)
            V.memset(cumX[0:1, 0:1], 0.0)

            def emit_statB(close=False):
                # start=True only on the first: it arms pending-zero for the
                # whole PSUM region, so the other columns lazily zero on
                # their first write (a second start would re-arm the region
                # and discard already-written columns)
                for j in range(4):
                    TE.matmul(pg[0:100, j:j + 1],
                              cst[0:101, C_STAT + j * 100:C_STAT + (j + 1) * 100],
                              h2rep[0:101, 0:1], start=(j == 0),
                              stop=(close and j == 3), skip_group_check=True)

            # step 0: h = 0 (h2rep DMA init) and the one-hot is all zero,
            # so the EW matmul is skipped and the group closes here
            emit_statB(close=True)

            prev_ctx = [None]

            G = nc.gpsimd

            def emit_offpath():
                px_ = prev_ctx[0]
                if px_ is None:
                    return
                k, step, Cs, T = px_
                if JK_DEP and step < 19:
                    # no-op rewrite of t_row[0,0] that reads the last one-hot
                    # block: pushes the jk dots behind the is_eq ops on the
                    # DVE queue so they stop polluting the critical window
                    V.scalar_tensor_tensor(t_row[0:1, 0:1], ohB[0:1, 0:1],
                                           0.0, t_row[0:1, 0:1],
                                           op0=OP.mult, op1=OP.add)
                # true e values (cum diff; the chains left e-1 in e_row)
                G.tensor_tensor(e_row[0:1, 0:k], cumX[0:1, 1:1 + k],
                                cumX[0:1, 0:k], op=OP.subtract)
                G.tensor_tensor(oh[0:1, 0:k], T[0:1, 0:k], T[0:1, 1:k + 1],
                                op=OP.subtract)
                G.tensor_copy(sumes[0:1, step:step + 1], cumX[0:1, k:k + 1])
                V.scalar_tensor_tensor(jk1[0:1, 0:k], oh[0:1, 0:k], 1.0,
                                       t_row[0:1, 0:k], op0=OP.mult,
                                       op1=OP.mult,
                                       accum_out=ta20[0:1, step:step + 1])
                V.scalar_tensor_tensor(jk2[0:1, 0:k], e_row[0:1, 0:k], 1.0,
                                       t_row[0:1, 0:k], op0=OP.mult,
                                       op1=OP.mult,
                                       accum_out=tsd20[0:1, step:step + 1])
                prev_ctx[0] = None

            def emit_step(kind, k, pos, step):
                Cs = TANH_C if kind == "node" else TANH_C / OP_TANH_REDUCE
                T = Tn if kind == "node" else To
                iota_col = C_IOTA_N if kind == "node" else C_IOTA_O

                # ---- finish LSTM gates: one block-diagonal one-hot MM ----
                if step > 0:
                    TE.matmul(pg[0:100, 0:4], cstA[0:128, 0:100],
                              ohB[0:128, 0:4], start=False, stop=True,
                              skip_group_check=True)

                # previous step's log_p/entropy bookkeeping runs on DVE
                # while PE/ACT work above
                emit_offpath()

                # ---- sigmoid-free LSTM (C = 2c, H = 2h) ----
                S.activation(th4[:, :], pg[0:100, 0:4], AF.Tanh)
                V.scalar_tensor_tensor(m1[:, :], th4[0:100, 0:1], 1.0,
                                       th4[0:100, 3:4], op0=OP.add,
                                       op1=OP.mult)   # P1 = (1+th_i)*th_g
                V.scalar_tensor_tensor(tg[:, :], th4[0:100, 1:2], 1.0,
                                       c[:, :], op0=OP.add,
                                       op1=OP.mult)   # P2 = (1+th_f)*C
                V.scalar_tensor_tensor(c[:, :], tg[:, :], 0.5, m1[:, :],
                                       op0=OP.mult, op1=OP.add)  # C'
                S.activation(tg[:, :], c[:, :], AF.Tanh, scale=0.5)
                V.scalar_tensor_tensor(
                    h2rep[0:100, 0:6],
                    th4[0:100, 2:3].broadcast_to([100, 6]), 1.0,
                    tg[0:100, 0:1].broadcast_to([100, 6]),
                    op0=OP.add, op1=OP.mult)  # H = (1+th_o)*tanh(C'/2)

                # ---- logits ----
                if kind == "node":
                    TE.matmul(psp[0:100, 0:k],
                              cst[0:100 + k, C_WHID:C_WHID + 100],
                              h2rep[0:100 + k, 0:k],
                              start=True, stop=True, skip_group_check=True)
                    S.activation(s[0:100, 0:k], psp[0:100, 0:k], AF.Tanh)
                    if DUMMY_DEP:
                        # rewrite cst[0,0] with its own value, reading s:
                        # creates a real s -> statB-weights dependency so the
                        # greedy PE schedule runs the logits matmul first
                        V.scalar_tensor_tensor(cst[0:1, 0:1], s[0:1, 0:1],
                                               0.0, cst[0:1, 0:1],
                                               op0=OP.mult, op1=OP.add)
                    TE.matmul(pl[0:1, 0:k], cst[0:100, C_V:C_V + 1],
                              s[0:100, 0:k], start=True, stop=True,
                              skip_group_check=True)
                else:
                    TE.matmul(pl[0:1, 0:8], h2rep[0:101, 0:1],
                              cst[0:101, C_WSOFT:C_WSOFT + 8],
                              start=True, stop=True, skip_group_check=True)

                # ---- prefetch next step's recurrent gate matmuls ----
                if step < 19:
                    emit_statB()

                # ---- softmax (log-space) + inverse-CDF sample ----
                S.activation(t_row[0:1, 0:k], pl[0:1, 0:k], AF.Tanh,
                             scale=1.0 / TEMP)
                # exp(Cs*t) - 1 via two interleaved Horner chains in t^2
                ev, od, q1 = _EXPC_NODE if kind == "node" else _EXPC_OP
                V.tensor_tensor(t2[0:1, 0:k], t_row[0:1, 0:k],
                                t_row[0:1, 0:k], op=OP.mult)
                V.tensor_scalar(pa[0:1, 0:k], t2[0:1, 0:k], ev[0], None,
                                op0=OP.mult)
                V.tensor_scalar(pc[0:1, 0:k], t2[0:1, 0:k], od[0], None,
                                op0=OP.mult)
                for i in range(1, max(len(ev), len(od))):
                    if i < len(ev):
                        V.scalar_tensor_tensor(pa[0:1, 0:k], pa[0:1, 0:k],
                                               ev[i], t2[0:1, 0:k],
                                               op0=OP.add, op1=OP.mult)
                    if i < len(od):
                        V.scalar_tensor_tensor(pc[0:1, 0:k], pc[0:1, 0:k],
                                               od[i], t2[0:1, 0:k],
                                               op0=OP.add, op1=OP.mult)
                V.scalar_tensor_tensor(px[0:1, 0:k], pc[0:1, 0:k], q1,
                                       t_row[0:1, 0:k], op0=OP.add,
                                       op1=OP.mult)
                V.tensor_tensor(e_row[0:1, 0:k], pa[0:1, 0:k], px[0:1, 0:k],
                                op=OP.add)  # e - 1
                # cumsum of (e-1) + 1 in one scan; col 0 stays 0
                V.tensor_tensor_scan(cumX[0:1, 1:1 + k], e_row[0:1, 0:k],
                                     cst[0:1, C_ONES:C_ONES + k], 0.0,
                                     op0=OP.add, op1=OP.add)
                # indicator cum_j * (1/u) < sum_e, with fused count
                V.scalar_tensor_tensor(
                    T[0:1, 1:k], cumX[0:1, 1:k],
                    cst[0:1, C_U + step:C_U + step + 1],
                    cumX[0:1, k:k + 1].broadcast_to([1, k - 1]),
                    op0=OP.mult, op1=OP.is_lt,
                    accum_out=outb[0:1, pos:pos + 1])

                # ---- fold action back in as block-diagonal one-hot ----
                if step < 19:
                    TE.matmul(pb[0:14, 0:1], cst[0:1, C_ONES:C_ONES + 14],
                              outb[0:1, pos:pos + 1], start=True, stop=True,
                              skip_group_check=True)
                    # block 0 last: the off-path dummy dep reads ohB[0,0]
                    for j in (1, 2, 3, 0):
                        V.tensor_tensor(ohB[32 * j:32 * j + 14, j:j + 1],
                                        cst[0:14, iota_col:iota_col + 1],
                                        pb[0:14, 0:1], op=OP.is_equal)

                prev_ctx[0] = (k, step, Cs, T)

            step = 0
            for node_idx in range(N_NODES):
                for i in range(2):
                    emit_step("node", node_idx + 2, node_idx * 2 + i, step)
                    step += 1
                for i in range(2):
                    emit_step("op", 8, 10 + node_idx * 2 + i, step)
                    step += 1
            emit_offpath()

            # log_p  = sum_s Cs*t_a[s] - sum_s ln(sume[s])
            # entropy = sum_s ln(sume[s]) - sum_s Cs*tsd[s]/sume[s]
            S.activation(lns[0:1, 0:20], sumes[0:1, 0:20], AF.Ln,
                         accum_out=lnt[0:1, 0:1])
            V.scalar_tensor_tensor(jk3[0:1, 0:20], ta20[0:1, 0:20], 1.0,
                                   cst[0:1, C_CSV:C_CSV + 20], op0=OP.mult,
                                   op1=OP.mult, accum_out=atp[0:1, 0:1])
            V.reciprocal(rec20[0:1, 0:20], sumes[0:1, 0:20])
            V.tensor_tensor(q20[0:1, 0:20], tsd20[0:1, 0:20],
                            rec20[0:1, 0:20], op=OP.mult)
            V.scalar_tensor_tensor(jk3[0:1, 0:20], q20[0:1, 0:20], 1.0,
                                   cst[0:1, C_CSV:C_CSV + 20], op0=OP.mult,
                                   op1=OP.mult, accum_out=enp[0:1, 0:1])
            V.tensor_tensor(outb[0:1, 20:21], atp[:, :], lnt[:, :],
                            op=OP.subtract)
            V.tensor_tensor(outb[0:1, 21:22], lnt[:, :], enp[:, :],
                            op=OP.subtract)

            nc.sync.dma_start(out=out_d[:, :], in_=outb[0:1, 0:24])

    nc.compile()
    return nc


_NC_CACHE = {}


def _get_nc():
    if "nc" not in _NC_CACHE:
        _NC_CACHE["nc"] = _build()
    return _NC_CACHE["nc"]


def _run(inputs, trace=False):
    from concourse.bass_utils import run_bass_kernel_spmd

    blobs = _host_constants(**{k: np.asarray(v, np.float32)
                               for k, v in inputs.items()})
    nc = _get_nc()
    core_ids = list(range(8))
    in_maps = [dict(blobs) for _ in core_ids]
    res = run_bass_kernel_spmd(nc, in_maps, core_ids, trace=trace)
    return res


def _unpack(out):
    out = np.asarray(out).reshape(-1)
    prev_nodes = np.rint(out[0:10]).astype(np.int32)
    prev_ops = np.rint(out[10:20]).astype(np.int32)
    log_p = np.float32(out[20])
    entropy = np.float32(out[21])
    return prev_nodes, prev_ops, log_p, entropy


def kernel(**inputs):
    res = _run(inputs, trace=False)
    return _unpack(res.results[0]["out"])
